# revision 22
# baseline (speedup 1.0000x reference)
"""CodeWiseAttention kernel for Trainium2 (8 NeuronCores, label-dim sharded).

m[b,n,:] = softmax(label_feature[n] @ x[b].T) @ x[b]

Sharding: label rows N=8922 split across 8 cores (1116/core; core 7 has
1110 real rows). x replicated.

v2 design (vs fp32r baseline):
  - mm1 in fp16 (1 cycle/row on PE vs ~3.3 for fp32 HIGH mode); scores
    accumulate in fp32 PSUM. fp16 input rounding keeps score error ~1e-3.
  - exp on ScalarE reads fp32 PSUM, writes bf16 (range needs bf16: e^30).
  - mm2 in bf16 (xa bf16 stationary, expS bf16 moving), fp32 PSUM accum.
  - No on-device input transposes: host supplies xT [E,L] fp16 and
    labT [E,N] fp16 directly; xa pre-chunked [126,20,101] bf16.
  - j-outer loop (3 n-chunks of 372); exp instructions span 3 l-chunks
    (free dim 1116) to amortize ScalarE's ~352-cycle fixed overhead.
  - s_ps double-buffered (2x3 PSUM banks) so PE never stalls on exp.

Per core, per batch, per n-chunk j (372 labels):
  for each group of 3 l-chunks (126 rows each; 7 groups cover 2520):
    mm1 x3: S^T[l,n] = xT[e,l].T @ labT[e,n]          (fp16, PSUM f32)
    exp:    e_sb[l, 3, n] = exp(S - 30) -> bf16        (one ACT instr)
    mm2 x3: U^T[e',n] += xa[l,e'].T @ e_sb[l,n]        (bf16, accum PSUM)
  xa has a ones column so row 100 of U^T = Z = sum_l expS.
  out: copy U^T to SBUF, PE-transpose 124-wide tiles, m = U/Z, DMA out.
"""
import numpy as np
from contextlib import ExitStack

import ml_dtypes

import concourse.tile as tile
from concourse import bacc, mybir
from concourse.bass_utils import run_bass_kernel_spmd

F32 = mybir.dt.float32
F16 = mybir.dt.float16
BF16 = mybir.dt.bfloat16

BF16NP = ml_dtypes.bfloat16

B, L, E = 8, 2500, 100
LP = 2520          # L padded (pad rows: xT cols zero, xa rows zero)
LC = 126           # l-chunk rows
NLC = LP // LC     # 20 l-chunks
CG = 3             # l-chunks per exp group
# short group FIRST: the boundary exp (last group of each j) must be a
# full 1116-wide instruction so it covers the PE chain mm2(last)+mm1(next)
# that gates the next j's first exp; a trailing 744-wide exp leaves a
# ~480ns ScalarE bubble at every j boundary.
GROUPS = [2] + [CG] * 6    # 2+18 = 20 l-chunks
HEADC = 2          # l-chunks in the head DMA piece (covers group 0)
N_TOTAL = 8922
NCORES = 8
NS = 1116          # label rows per core (core 7: 1110 real); 3*372
NCH = 372          # n-chunk width (>=256 keeps matmul at full rate)
NJ = NS // NCH     # 3 n-chunks
NO = 124           # out-tile rows; 9*124 = 1116
EA = E + 1         # x augmented with ones column
PSB = 512          # psum bank stride in f32 elements
EXP_BIAS = -30.0

TRACE = False
LAST_RESULT = None

_NC = []


def _build():
    nc = bacc.Bacc("TRN2", target_bir_lowering=False, debug=False)
    xt_d = nc.dram_tensor("xt", [B, E, LP], F16, kind="ExternalInput").ap()
    xa_d = nc.dram_tensor("xa", [B, LC, NLC, EA], BF16, kind="ExternalInput").ap()
    labt_d = nc.dram_tensor("labt", [E, NS], F16, kind="ExternalInput").ap()
    idf_d = nc.dram_tensor("idf", [128, 128], F32, kind="ExternalInput").ap()
    m_d = nc.dram_tensor("m", [B, NS, E], F32, kind="ExternalOutput").ap()
    # raw U^T/Z for the very last (b, j) block; normalizing + transposing it
    # on-device would serialize ~3.5us after the final exp, so the host
    # finishes that one block instead
    uraw_d = nc.dram_tensor("uraw", [EA, NCH], F32, kind="ExternalOutput").ap()

    with tile.TileContext(nc) as tc, ExitStack() as ctx:
        consts = ctx.enter_context(tc.tile_pool(name="consts", bufs=1))
        xt_pool = ctx.enter_context(tc.tile_pool(name="xtp", bufs=3))
        xa_pool = ctx.enter_context(tc.tile_pool(name="xap", bufs=3))
        e_pool = ctx.enter_context(tc.tile_pool(name="ep", bufs=5))
        u_pool = ctx.enter_context(tc.tile_pool(name="up", bufs=2))
        o_pool = ctx.enter_context(tc.tile_pool(name="op", bufs=4))
        r_pool = ctx.enter_context(tc.tile_pool(name="rp", bufs=4))
        pss = ctx.enter_context(tc.tile_pool(name="pss", bufs=2, space="PSUM"))
        psm = ctx.enter_context(tc.tile_pool(name="psm", bufs=1, space="PSUM"))
        pstr = ctx.enter_context(tc.tile_pool(name="pstr", bufs=1, space="PSUM"))

        bias_sb = consts.tile([128, 1], F32)
        nc.vector.memset(bias_sb[:], EXP_BIAS)

        # xT piece boundaries in l-chunks; batch 0's pipeline starts after
        # the first ~125KB lands instead of the full ~1MB. Each DMA costs
        # ~0.7us of descriptor generation on the sync queue, so the
        # critical-path pieces (xT head, labt j0, xa head) are issued first.
        XT_CUTS = [0, HEADC, 6, 12, NLC]
        xt_tiles = {}
        xa_tiles = {}
        labt_sb = []

        def fetch(b):
            pieces = []
            for pi in range(len(XT_CUTS) - 1):
                lo, hi = XT_CUTS[pi], XT_CUTS[pi + 1]
                xp = xt_pool.tile([E, (hi - lo) * LC], F16, tag=f"xt{pi}",
                                  name=f"xt{pi}_{b}")
                nc.sync.dma_start(out=xp[:], in_=xt_d[b, :, lo * LC:hi * LC])
                pieces.append(xp)
                if b == 0 and pi == 0:
                    lt0 = consts.tile([E, NCH], F16, name="labt0")
                    nc.sync.dma_start(out=lt0[:], in_=labt_d[:, 0:NCH])
                    labt_sb.append(lt0)
                    xah = xa_pool.tile([LC, HEADC, EA], BF16, tag="xah",
                                       name=f"xah{b}")
                    nc.sync.dma_start(out=xah[:], in_=xa_d[b, :, 0:HEADC, :])
            if b > 0:
                xah = xa_pool.tile([LC, HEADC, EA], BF16, tag="xah",
                                   name=f"xah{b}")
                nc.sync.dma_start(out=xah[:], in_=xa_d[b, :, 0:HEADC, :])
            xat = xa_pool.tile([LC, NLC - HEADC, EA], BF16, tag="xat",
                              name=f"xat{b}")
            nc.sync.dma_start(out=xat[:], in_=xa_d[b, :, HEADC:NLC, :])
            if b == 0:
                for j in range(1, NJ):
                    lt = consts.tile([E, NCH], F16, name=f"labt{j}")
                    nc.sync.dma_start(
                        out=lt[:], in_=labt_d[:, j * NCH:(j + 1) * NCH])
                    labt_sb.append(lt)
            xt_tiles[b] = pieces
            xa_tiles[b] = (xah, xat)

        def xt_col(tiles, c):
            for pi in range(len(XT_CUTS) - 1):
                if c < XT_CUTS[pi + 1]:
                    cc = c - XT_CUTS[pi]
                    return tiles[pi][:, cc * LC:(cc + 1) * LC]
            raise AssertionError

        def xa_row(tiles, c):
            if c < HEADC:
                return tiles[0][:, c, :]
            return tiles[1][:, c - HEADC, :]

        # Out-path work for the just-finished (b, j) is interleaved into the
        # NEXT j's groups so its PE transposes never sit in the PE queue
        # ahead of the next mm1 block (which would stall the exp pipeline).
        pending = []   # list of closures, one 124-wide out tile each

        def out_path(b, j, u_sb, t, pool=None):
            def emit():
                tpo = (pool or pstr).tile([128, 128], F32, tag="tr",
                                          name="tpo")
                nc.tensor.transpose(
                    tpo[:NO, :EA], u_sb[:, t * NO:(t + 1) * NO],
                    idf_sb[:EA, :EA],
                )
                rz = r_pool.tile([NO, 1], F32, tag="r")
                nc.vector.reciprocal(rz[:], tpo[:NO, E:EA])
                o_sb = o_pool.tile([NO, E], F32, tag="o")
                nc.vector.tensor_scalar_mul(o_sb[:], tpo[:NO, 0:E], rz[:])
                n0 = j * NCH + t * NO
                nc.sync.dma_start(out=m_d[b, n0:n0 + NO, :], in_=o_sb[:])
            return emit

        fetch(0)
        idf_sb = consts.tile([128, 128], F32)
        nc.sync.dma_start(out=idf_sb[:], in_=idf_d)

        # Flat software pipeline over all (b, j, group) items. The mm2 block
        # of group G is issued TWO groups after its exp: every PE instruction
        # preceding exp(G+1) in program order (and hence inside its semaphore
        # threshold) then completes at least one full exp earlier, so exps
        # chain back-to-back on ScalarE with no exp->mm2->exp serial bubble.
        items = []
        for b in range(B):
            for j in range(NJ):
                c = 0
                for gi, cg in enumerate(GROUPS):
                    items.append((b, j, gi, cg, c))
                    c += cg

        m_ps_cur = [None]   # current j's accumulator psum tile

        def issue_mm2(it2):
            b2, j2, gi2, cg2, c2 = it2
            if gi2 == 0:
                m_ps_cur[0] = psm.tile([EA, PSB], F32, tag="m", name="m_ps")
            m_ps = m_ps_cur[0]
            xa_sb = xa_tiles[b2]
            e_sb = e_tiles.pop((b2, j2, gi2))
            for k in range(cg2):
                nc.tensor.matmul(
                    m_ps[:, 0:NCH],
                    xa_row(xa_sb, c2 + k),
                    e_sb[:, k, :],
                    start=(c2 + k == 0), stop=(c2 + k == NLC - 1),
                )
            if gi2 == len(GROUPS) - 1:
                u_sb = u_pool.tile([EA, NCH], F32, tag="u")
                nc.vector.tensor_copy(u_sb[:], m_ps[:, 0:NCH])
                if b2 == B - 1 and j2 == NJ - 1:
                    nc.sync.dma_start(out=uraw_d, in_=u_sb[:])
                else:
                    pending.extend(
                        out_path(b2, j2, u_sb, t) for t in range(NCH // NO))

        e_tiles = {}
        mm2_q = []
        for it in items + [None, None, None]:
            if it is not None:
                b, j, gi, cg, c = it
                if j == 0 and gi == 0 and b + 1 < B:
                    fetch(b + 1)
                xt_sb = xt_tiles[b]
                s_ps = pss.tile([LC, CG, PSB], F32, tag="s")
                for k in range(cg):
                    nc.tensor.matmul(
                        s_ps[:, k, 0:NCH],
                        xt_col(xt_sb, c + k),
                        labt_sb[j][:, :],
                    )
                e_sb = e_pool.tile([LC, CG, NCH], BF16, tag="e")
                nc.scalar.activation(
                    e_sb[:, 0:cg, :], s_ps[:, 0:cg, 0:NCH],
                    mybir.ActivationFunctionType.Exp,
                    bias=bias_sb[:LC], scale=1.0,
                )
                e_tiles[(b, j, gi)] = e_sb
                mm2_q.append(it)
            if (len(mm2_q) > 3) or (it is None and mm2_q):
                issue_mm2(mm2_q.pop(0))
                if pending:
                    pending.pop(0)()
        for p in pending:
            p()
    nc.compile()
    return nc


def _get_nc():
    if not _NC:
        _NC.append(_build())
    return _NC[0]


def kernel(x, label_feature):
    global LAST_RESULT
    x = np.ascontiguousarray(np.asarray(x, dtype=np.float32))
    lf = np.ascontiguousarray(np.asarray(label_feature, dtype=np.float32))
    assert x.shape == (B, L, E) and lf.shape == (N_TOTAL, E)

    # xT [B, E, LP] fp16 (mm1 stationary source; pad cols zero)
    xt = np.zeros((B, E, LP), np.float16)
    xt[:, :, :L] = x.transpose(0, 2, 1)
    # xa [B, LP, EA] bf16 with ones column, pre-chunked to [B, LC, NLC, EA]
    xa_full = np.zeros((B, LP, EA), np.float32)
    xa_full[:, :L, :E] = x
    xa_full[:, :L, E] = 1.0
    xa = np.ascontiguousarray(
        xa_full.reshape(B, NLC, LC, EA).transpose(0, 2, 1, 3)
    ).astype(BF16NP)
    ident = np.eye(128, dtype=np.float32)

    in_maps = []
    for r in range(NCORES):
        lo = r * NS
        hi = min(lo + NS, N_TOTAL)
        shard = np.zeros((NS, E), np.float32)
        shard[: hi - lo] = lf[lo:hi]
        labt = np.ascontiguousarray(shard.T).astype(np.float16)
        in_maps.append({"xt": xt, "xa": xa, "labt": labt, "idf": ident})

    nc = _get_nc()
    res = run_bass_kernel_spmd(
        nc, in_maps, core_ids=list(range(NCORES)), trace=TRACE
    )
    LAST_RESULT = res

    out = np.empty((B, N_TOTAL, E), np.float32)
    for r in range(NCORES):
        lo = r * NS
        hi = min(lo + NS, N_TOTAL)
        m = res.results[r]["m"][:, : hi - lo, :].copy()
        # device skips the out-path for the very last (b, j) block; finish
        # it here from the raw U^T/Z dump
        uraw = res.results[r]["uraw"].astype(np.float64)
        blk = (uraw[:E] / uraw[E]).T.astype(np.float32)   # [NCH, E]
        n0 = (NJ - 1) * NCH
        take = min(NCH, hi - lo - n0)
        if take > 0:
            m[B - 1, n0:n0 + take, :] = blk[:take]
        out[:, lo:hi, :] = m
    return out


# revision 26
# speedup vs baseline: 1.0236x; 1.0236x over previous
"""CodeWiseAttention kernel for Trainium2 (8 NeuronCores, label-dim sharded).

m[b,n,:] = softmax(label_feature[n] @ x[b].T) @ x[b]

Sharding: label rows N=8922 split across 8 cores (1116/core; core 7 has
1110 real rows). x replicated.

v2 design (vs fp32r baseline):
  - mm1 in fp16 (1 cycle/row on PE vs ~3.3 for fp32 HIGH mode); scores
    accumulate in fp32 PSUM. fp16 input rounding keeps score error ~1e-3.
  - exp on ScalarE reads fp32 PSUM, writes bf16 (range needs bf16: e^30).
  - mm2 in bf16 (xa bf16 stationary, expS bf16 moving), fp32 PSUM accum.
  - No on-device input transposes: host supplies xT [E,L] fp16 and
    labT [E,N] fp16 directly; xa pre-chunked [126,20,101] bf16.
  - j-outer loop (3 n-chunks of 372); exp instructions span 3 l-chunks
    (free dim 1116) to amortize ScalarE's ~352-cycle fixed overhead.
  - s_ps double-buffered (2x3 PSUM banks) so PE never stalls on exp.

Per core, per batch, per n-chunk j (372 labels):
  for each group of 3 l-chunks (126 rows each; 7 groups cover 2520):
    mm1 x3: S^T[l,n] = xT[e,l].T @ labT[e,n]          (fp16, PSUM f32)
    exp:    e_sb[l, 3, n] = exp(S - 30) -> bf16        (one ACT instr)
    mm2 x3: U^T[e',n] += xa[l,e'].T @ e_sb[l,n]        (bf16, accum PSUM)
  xa has a ones column so row 100 of U^T = Z = sum_l expS.
  out: copy U^T to SBUF, PE-transpose 124-wide tiles, m = U/Z, DMA out.
"""
import numpy as np
from contextlib import ExitStack

import ml_dtypes

import concourse.tile as tile
from concourse import bacc, mybir
from concourse.bass_utils import run_bass_kernel_spmd

F32 = mybir.dt.float32
F16 = mybir.dt.float16
BF16 = mybir.dt.bfloat16

BF16NP = ml_dtypes.bfloat16

B, L, E = 8, 2500, 100
LP = 2520          # L padded (pad rows: xT cols zero, xa rows zero)
LC = 126           # l-chunk rows
NLC = LP // LC     # 20 l-chunks
CG = 3             # l-chunks per exp group
# short group FIRST: the boundary exp (last group of each j) must be a
# full 1116-wide instruction so it covers the PE chain mm2(last)+mm1(next)
# that gates the next j's first exp; a trailing 744-wide exp leaves a
# ~480ns ScalarE bubble at every j boundary.
GROUPS = [2] + [CG] * 6    # 2+18 = 20 l-chunks
HEADC = 2          # l-chunks in the head DMA piece (covers group 0)
N_TOTAL = 8922
NCORES = 8
NS = 1116          # label rows per core (core 7: 1110 real); 3*372
NCH = 372          # n-chunk width (>=256 keeps matmul at full rate)
NJ = NS // NCH     # 3 n-chunks
NO = 124           # out-tile rows; 9*124 = 1116
EA = E + 1         # x augmented with ones column
PSB = 512          # psum bank stride in f32 elements
EXP_BIAS = -30.0

TRACE = False
LAST_RESULT = None

_NC = []


def _build():
    nc = bacc.Bacc("TRN2", target_bir_lowering=False, debug=False)
    xt_d = nc.dram_tensor("xt", [B, E, LP], F16, kind="ExternalInput").ap()
    xa_d = nc.dram_tensor("xa", [B, LC, NLC, EA], BF16, kind="ExternalInput").ap()
    labt_d = nc.dram_tensor("labt", [E, NS], F16, kind="ExternalInput").ap()
    idf_d = nc.dram_tensor("idf", [128, 128], F32, kind="ExternalInput").ap()
    m_d = nc.dram_tensor("m", [B, NS, E], F32, kind="ExternalOutput").ap()

    with tile.TileContext(nc) as tc, ExitStack() as ctx:
        consts = ctx.enter_context(tc.tile_pool(name="consts", bufs=1))
        xt_pool = ctx.enter_context(tc.tile_pool(name="xtp", bufs=3))
        xa_pool = ctx.enter_context(tc.tile_pool(name="xap", bufs=3))
        e_pool = ctx.enter_context(tc.tile_pool(name="ep", bufs=5))
        u_pool = ctx.enter_context(tc.tile_pool(name="up", bufs=2))
        o_pool = ctx.enter_context(tc.tile_pool(name="op", bufs=4))
        r_pool = ctx.enter_context(tc.tile_pool(name="rp", bufs=4))
        pss = ctx.enter_context(tc.tile_pool(name="pss", bufs=2, space="PSUM"))
        psm = ctx.enter_context(tc.tile_pool(name="psm", bufs=1, space="PSUM"))
        pstr = ctx.enter_context(tc.tile_pool(name="pstr", bufs=1, space="PSUM"))

        bias_sb = consts.tile([128, 1], F32)
        nc.vector.memset(bias_sb[:], EXP_BIAS)

        # xT piece boundaries in l-chunks; batch 0's pipeline starts after
        # the first ~125KB lands instead of the full ~1MB. Each DMA costs
        # ~0.7us of descriptor generation on the sync queue, so the
        # critical-path pieces (xT head, labt j0, xa head) are issued first.
        XT_CUTS = [0, HEADC, 10, NLC]
        xt_tiles = {}
        xa_tiles = {}
        labt_sb = []

        def fetch(b):
            pieces = []
            for pi in range(len(XT_CUTS) - 1):
                lo, hi = XT_CUTS[pi], XT_CUTS[pi + 1]
                xp = xt_pool.tile([E, (hi - lo) * LC], F16, tag=f"xt{pi}",
                                  name=f"xt{pi}_{b}")
                nc.sync.dma_start(out=xp[:], in_=xt_d[b, :, lo * LC:hi * LC])
                pieces.append(xp)
                if b == 0 and pi == 0:
                    lt0 = consts.tile([E, NCH], F16, name="labt0")
                    nc.sync.dma_start(out=lt0[:], in_=labt_d[:, 0:NCH])
                    labt_sb.append(lt0)
                    xah = xa_pool.tile([LC, HEADC, EA], BF16, tag="xah",
                                       name=f"xah{b}")
                    nc.sync.dma_start(out=xah[:], in_=xa_d[b, :, 0:HEADC, :])
            if b > 0:
                xah = xa_pool.tile([LC, HEADC, EA], BF16, tag="xah",
                                   name=f"xah{b}")
                nc.sync.dma_start(out=xah[:], in_=xa_d[b, :, 0:HEADC, :])
            xat = xa_pool.tile([LC, NLC - HEADC, EA], BF16, tag="xat",
                              name=f"xat{b}")
            nc.sync.dma_start(out=xat[:], in_=xa_d[b, :, HEADC:NLC, :])
            if b == 0:
                for j in range(1, NJ):
                    lt = consts.tile([E, NCH], F16, name=f"labt{j}")
                    nc.sync.dma_start(
                        out=lt[:], in_=labt_d[:, j * NCH:(j + 1) * NCH])
                    labt_sb.append(lt)
            xt_tiles[b] = pieces
            xa_tiles[b] = (xah, xat)

        def xt_col(tiles, c):
            for pi in range(len(XT_CUTS) - 1):
                if c < XT_CUTS[pi + 1]:
                    cc = c - XT_CUTS[pi]
                    return tiles[pi][:, cc * LC:(cc + 1) * LC]
            raise AssertionError

        def xa_row(tiles, c):
            if c < HEADC:
                return tiles[0][:, c, :]
            return tiles[1][:, c - HEADC, :]

        # Out-path work for the just-finished (b, j) is interleaved into the
        # NEXT j's groups so its PE transposes never sit in the PE queue
        # ahead of the next mm1 block (which would stall the exp pipeline).
        pending = []   # list of closures, one 124-wide out tile each

        def out_path(b, j, u_sb, t, pool=None):
            def emit():
                tpo = (pool or pstr).tile([128, 128], F32, tag="tr",
                                          name="tpo")
                nc.tensor.transpose(
                    tpo[:NO, :EA], u_sb[:, t * NO:(t + 1) * NO],
                    idf_sb[:EA, :EA],
                )
                rz = r_pool.tile([NO, 1], F32, tag="r")
                nc.vector.reciprocal(rz[:], tpo[:NO, E:EA])
                o_sb = o_pool.tile([NO, E], F32, tag="o")
                nc.vector.tensor_scalar_mul(o_sb[:], tpo[:NO, 0:E], rz[:])
                n0 = j * NCH + t * NO
                nc.sync.dma_start(out=m_d[b, n0:n0 + NO, :], in_=o_sb[:])
            return emit

        fetch(0)
        idf_sb = consts.tile([128, 128], F32)
        nc.sync.dma_start(out=idf_sb[:], in_=idf_d)

        # Flat software pipeline over all (b, j, group) items. The mm2 block
        # of group G is issued TWO groups after its exp: every PE instruction
        # preceding exp(G+1) in program order (and hence inside its semaphore
        # threshold) then completes at least one full exp earlier, so exps
        # chain back-to-back on ScalarE with no exp->mm2->exp serial bubble.
        items = []
        for b in range(B):
            for j in range(NJ):
                c = 0
                for gi, cg in enumerate(GROUPS):
                    items.append((b, j, gi, cg, c))
                    c += cg

        m_ps_cur = [None]   # current j's accumulator psum tile

        def issue_mm2(it2):
            b2, j2, gi2, cg2, c2 = it2
            if gi2 == 0:
                m_ps_cur[0] = psm.tile([EA, PSB], F32, tag="m", name="m_ps")
            m_ps = m_ps_cur[0]
            xa_sb = xa_tiles[b2]
            e_sb = e_tiles.pop((b2, j2, gi2))
            for k in range(cg2):
                nc.tensor.matmul(
                    m_ps[:, 0:NCH],
                    xa_row(xa_sb, c2 + k),
                    e_sb[:, k, :],
                    start=(c2 + k == 0), stop=(c2 + k == NLC - 1),
                )
            if gi2 == len(GROUPS) - 1:
                u_sb = u_pool.tile([EA, NCH], F32, tag="u")
                nc.vector.tensor_copy(u_sb[:], m_ps[:, 0:NCH])
                pending.extend(
                    out_path(b2, j2, u_sb, t) for t in range(NCH // NO))

        e_tiles = {}
        mm2_q = []
        for it in items + [None, None, None]:
            if it is not None:
                b, j, gi, cg, c = it
                if j == 0 and gi == 0 and b + 1 < B:
                    fetch(b + 1)
                xt_sb = xt_tiles[b]
                s_ps = pss.tile([LC, CG, PSB], F32, tag="s")
                for k in range(cg):
                    nc.tensor.matmul(
                        s_ps[:, k, 0:NCH],
                        xt_col(xt_sb, c + k),
                        labt_sb[j][:, :],
                    )
                e_sb = e_pool.tile([LC, CG, NCH], BF16, tag="e")
                nc.scalar.activation(
                    e_sb[:, 0:cg, :], s_ps[:, 0:cg, 0:NCH],
                    mybir.ActivationFunctionType.Exp,
                    bias=bias_sb[:LC], scale=1.0,
                )
                e_tiles[(b, j, gi)] = e_sb
                mm2_q.append(it)
            if (len(mm2_q) > 3) or (it is None and mm2_q):
                issue_mm2(mm2_q.pop(0))
                if pending:
                    pending.pop(0)()
        for p in pending:
            p()
    nc.compile()
    return nc


def _get_nc():
    if not _NC:
        _NC.append(_build())
    return _NC[0]


def kernel(x, label_feature):
    global LAST_RESULT
    x = np.ascontiguousarray(np.asarray(x, dtype=np.float32))
    lf = np.ascontiguousarray(np.asarray(label_feature, dtype=np.float32))
    assert x.shape == (B, L, E) and lf.shape == (N_TOTAL, E)

    # xT [B, E, LP] fp16 (mm1 stationary source; pad cols zero)
    xt = np.zeros((B, E, LP), np.float16)
    xt[:, :, :L] = x.transpose(0, 2, 1)
    # xa [B, LP, EA] bf16 with ones column, pre-chunked to [B, LC, NLC, EA]
    xa_full = np.zeros((B, LP, EA), np.float32)
    xa_full[:, :L, :E] = x
    xa_full[:, :L, E] = 1.0
    xa = np.ascontiguousarray(
        xa_full.reshape(B, NLC, LC, EA).transpose(0, 2, 1, 3)
    ).astype(BF16NP)
    ident = np.eye(128, dtype=np.float32)

    in_maps = []
    for r in range(NCORES):
        lo = r * NS
        hi = min(lo + NS, N_TOTAL)
        shard = np.zeros((NS, E), np.float32)
        shard[: hi - lo] = lf[lo:hi]
        labt = np.ascontiguousarray(shard.T).astype(np.float16)
        in_maps.append({"xt": xt, "xa": xa, "labt": labt, "idf": ident})

    nc = _get_nc()
    res = run_bass_kernel_spmd(
        nc, in_maps, core_ids=list(range(NCORES)), trace=TRACE
    )
    LAST_RESULT = res

    out = np.empty((B, N_TOTAL, E), np.float32)
    for r in range(NCORES):
        lo = r * NS
        hi = min(lo + NS, N_TOTAL)
        out[:, lo:hi, :] = res.results[r]["m"][:, : hi - lo, :]
    return out
